# revision 1
# baseline (speedup 1.0000x reference)
"""HBMP (3-branch LSTM + BiLSTM + global max pool) Trainium2 kernel.

Model (B=64, T=512, E=300, H=512, NB=3 branches):
  per branch: h1 = LSTM(x); hf = LSTM(h1); hb = rev(LSTM(rev(h1)))
  emb = maxpool_T(concat([hf, hb], -1));  out = concat over branches [B, 3*2H]

Mapping onto 6 NeuronCores (task-parallel; batch stays whole because the
recurrent matmul cost is weight-streaming-bound, independent of batch):
  core c in 0..5 handles (branch = c%3, direction = fwd if c<3 else bwd):
    P0: xz_u = x @ Wx_u + b_u                  (dense matmul, M-tiled)
    P1: uni LSTM scan -> transposed h stream hT to DRAM
    P2: xz_d = h1 @ Wx_d + b_d                 (dense matmul over hT tiles;
        written T-REVERSED via indirect-DMA scatter for bwd cores, so one
        SPMD program serves both directions - direction lives in the
        per-core scatter-index table input)
    P3: dir LSTM scan over xz_d with running max -> rmax [64, 512]
Host gathers the 6 rmax outputs into [64, 3072].

Scan step: z (PSUM, [64, 4H]) accumulates xz_t (via identity matmul) plus
h_{t-1} @ Wh (4 K-tile matmuls with the transposed state hT as stationary);
gates on ScalarE from PSUM; c/h updates on VectorE; h re-transposed on PE.
"""
import sys

sys.path.insert(0, "/opt/trn_rl_repo")

import numpy as np

B, T, E, H = 64, 512, 300, 512
FOUR_H = 4 * H
NB = 3

_CACHE = {}


def _build_program(rep=1):
    import concourse.bass as bass
    import concourse.tile as tile
    from concourse import bacc, mybir

    F32 = mybir.dt.float32
    I32 = mybir.dt.int32
    Sig = mybir.ActivationFunctionType.Sigmoid
    Tanh = mybir.ActivationFunctionType.Tanh

    nc = bacc.Bacc("TRN2", target_bir_lowering=False, debug=False,
                   enable_asserts=False, num_devices=6)

    d = {}
    d["xTu"] = nc.dram_tensor("xTu", [T, 128, 3, 64], F32, kind="ExternalInput").ap()
    d["wxu"] = nc.dram_tensor("wxu", [128, 3, FOUR_H], F32, kind="ExternalInput").ap()
    d["whu"] = nc.dram_tensor("whu", [128, 4, FOUR_H], F32, kind="ExternalInput").ap()
    d["bu"] = nc.dram_tensor("bu", [128, FOUR_H], F32, kind="ExternalInput").ap()
    d["wxd"] = nc.dram_tensor("wxd", [128, 4, FOUR_H], F32, kind="ExternalInput").ap()
    d["whd"] = nc.dram_tensor("whd", [128, 4, FOUR_H], F32, kind="ExternalInput").ap()
    d["bd"] = nc.dram_tensor("bd", [128, FOUR_H], F32, kind="ExternalInput").ap()
    d["id64"] = nc.dram_tensor("id64", [64, 64], F32, kind="ExternalInput").ap()
    d["scat"] = nc.dram_tensor("scat", [128, T // 2], I32, kind="ExternalInput").ap()
    d["xzu"] = nc.dram_tensor("xzu", [T, B, FOUR_H], F32, kind="Internal").ap()
    d["hT"] = nc.dram_tensor("hT", [T, 128, 256], F32, kind="Internal").ap()
    d["xzd"] = nc.dram_tensor("xzd", [T, B, FOUR_H], F32, kind="Internal").ap()
    d["rmax"] = nc.dram_tensor("rmax", [B, H], F32, kind="ExternalOutput").ap()

    def build_xproj(tc):
        with (
            tc.tile_pool(name="p0w", bufs=1) as wp,
            tc.tile_pool(name="p0io", bufs=4) as iop,
            tc.tile_pool(name="p0ps", bufs=2, space="PSUM") as psp,
        ):
            wx_sb = wp.tile([128, 3, FOUR_H], F32, tag="wx")
            nc.sync.dma_start(wx_sb[:], d["wxu"])
            b_sb = wp.tile([128, FOUR_H], F32, tag="b")
            nc.sync.dma_start(b_sb[:], d["bu"])
            for m in range(T // 2):
                xt = iop.tile([128, 3, 2, 64], F32, tag="xt")
                nc.sync.dma_start(
                    xt[:], d["xTu"][2 * m:2 * m + 2].rearrange("t p k b -> p k t b"))
                zp = psp.tile([128, FOUR_H], F32, tag="zp")
                for k in range(3):
                    for n in range(4):
                        nc.tensor.matmul(
                            zp[:, bass.ts(n, 512)],
                            xt[:, k, :, :].rearrange("p t b -> p (t b)"),
                            wx_sb[:, k, bass.ts(n, 512)],
                            start=(k == 0), stop=(k == 2))
                zs = iop.tile([128, FOUR_H], F32, tag="zs")
                nc.vector.tensor_add(zs[:], zp[:], b_sb[:])
                nc.sync.dma_start(
                    d["xzu"][2 * m:2 * m + 2].rearrange("t b n -> (t b) n"), zs[:])

    def build_scan(tc, xz, wh_name, store_hT, rmax_out):
        # gate column order is host-permuted to [f i o g]:
        #   chunks: n0=f, n1=i, n2=o, n3=g
        with (
            tc.tile_pool(name=f"w_{wh_name}", bufs=1) as whp,
            tc.tile_pool(name=f"st_{wh_name}", bufs=1) as statep,
            tc.tile_pool(name=f"xz_{wh_name}", bufs=4) as xzp,
            tc.tile_pool(name=f"g_{wh_name}", bufs=2) as gp,
            tc.tile_pool(name=f"zps_{wh_name}", bufs=1, space="PSUM") as zpsp,
            tc.tile_pool(name=f"tps_{wh_name}", bufs=2, space="PSUM") as tpsp,
        ):
            wh_sb = whp.tile([128, 4, FOUR_H], F32, tag="wh")
            nc.sync.dma_start(wh_sb[:], d[wh_name])
            id_sb = whp.tile([64, 64], F32, tag="id")
            nc.sync.dma_start(id_sb[:], d["id64"])

            hT_sb = statep.tile([128, 4, 64], F32, tag="hT")
            # st = [c | tanh(g)] adjacent so one DVE mul makes [f*c | i*tg]
            st_sb = statep.tile([64, 2 * H], F32, tag="st")
            nc.vector.memset(hT_sb[:], 0.0)
            nc.vector.memset(st_sb[:], 0.0)
            if rmax_out is not None:
                rmax_sb = statep.tile([64, H], F32, tag="rmax")
                nc.vector.memset(rmax_sb[:], -1e30)

            for t in range(T):
                xz_t = xzp.tile([64, FOUR_H], F32, tag="xzt")
                nc.sync.dma_start(xz_t[:], xz[t])
                z = zpsp.tile([64, FOUR_H], F32, tag="z")
                for k in range(4):
                    for n in range(4):
                        nc.tensor.matmul(z[:, bass.ts(n, 512)], hT_sb[:, k, :],
                                         wh_sb[:, k, bass.ts(n, 512)],
                                         start=(k == 0), stop=(k == 3))
                zf = gp.tile([64, FOUR_H], F32, tag="zf")
                nc.vector.tensor_add(zf[:], z[:], xz_t[:])
                ga = gp.tile([64, 3 * H], F32, tag="ga")  # [sf si so]
                nc.scalar.activation(ga[:], zf[:, 0:3 * H], Sig)
                nc.scalar.activation(st_sb[:, H:2 * H], zf[:, 3 * H:4 * H], Tanh)
                t12 = gp.tile([64, 2 * H], F32, tag="t12")
                nc.vector.tensor_mul(t12[:], ga[:, 0:2 * H], st_sb[:])
                nc.vector.tensor_add(st_sb[:, 0:H], t12[:, 0:H], t12[:, H:2 * H])
                tc_t = gp.tile([64, H], F32, tag="tc")
                nc.scalar.activation(tc_t[:], st_sb[:, 0:H], Tanh)
                h_t = gp.tile([64, H], F32, tag="h")
                nc.vector.tensor_mul(h_t[:], ga[:, 2 * H:3 * H], tc_t[:])
                if rmax_out is not None:
                    nc.vector.tensor_max(rmax_sb[:], rmax_sb[:], h_t[:])
                pT = tpsp.tile([128, 4, 64], F32, tag="pT")
                for k in range(4):
                    nc.tensor.transpose(pT[:, k, :], h_t[:, bass.ts(k, 128)], id_sb[:])
                nc.vector.tensor_copy(hT_sb[:], pT[:])
                if store_hT:
                    nc.sync.dma_start(d["hT"][t],
                                      hT_sb[:].rearrange("p k b -> p (k b)"))
            if rmax_out is not None:
                nc.sync.dma_start(rmax_out, rmax_sb[:])

    def build_hproj(tc):
        with (
            tc.tile_pool(name="p2w", bufs=1) as wp,
            tc.tile_pool(name="p2io", bufs=4) as iop,
            tc.tile_pool(name="p2ps", bufs=2, space="PSUM") as psp,
        ):
            wx_sb = wp.tile([128, 4, FOUR_H], F32, tag="wx")
            nc.sync.dma_start(wx_sb[:], d["wxd"])
            b_sb = wp.tile([128, FOUR_H], F32, tag="b")
            nc.sync.dma_start(b_sb[:], d["bd"])
            scat_sb = wp.tile([128, T // 2], I32, tag="scat")
            nc.sync.dma_start(scat_sb[:], d["scat"])
            xzd_rows = d["xzd"].rearrange("t b n -> (t b) n")
            for m in range(T // 2):
                ht = iop.tile([128, 4, 2, 64], F32, tag="ht")
                nc.sync.dma_start(
                    ht[:],
                    d["hT"][2 * m:2 * m + 2].rearrange("t p (k b) -> p k t b", k=4))
                zp = psp.tile([128, FOUR_H], F32, tag="zp")
                for k in range(4):
                    for n in range(4):
                        nc.tensor.matmul(
                            zp[:, bass.ts(n, 512)],
                            ht[:, k, :, :].rearrange("p t b -> p (t b)"),
                            wx_sb[:, k, bass.ts(n, 512)],
                            start=(k == 0), stop=(k == 3))
                zs = iop.tile([128, FOUR_H], F32, tag="zs")
                nc.vector.tensor_add(zs[:], zp[:], b_sb[:])
                nc.gpsimd.indirect_dma_start(
                    out=xzd_rows,
                    out_offset=bass.IndirectOffsetOnAxis(
                        ap=scat_sb[:, m:m + 1], axis=0),
                    in_=zs[:],
                    in_offset=None)

    with tile.TileContext(nc) as tc:
        for _ in range(rep):
            build_xproj(tc)
            build_scan(tc, d["xzu"], "whu", store_hT=True, rmax_out=None)
            build_hproj(tc)
            build_scan(tc, d["xzd"], "whd", store_hT=False, rmax_out=d["rmax"])
    nc.compile()
    return nc


def _prep_shared(x):
    """x [B,T,E] -> xT [T,128,3,64] with xT[t,p,k,b] = x[b,t,k*128+p] (E pad 384)."""
    xpad = np.zeros((B, T, 384), np.float32)
    xpad[:, :, :E] = x
    xT = xpad.transpose(1, 2, 0).reshape(T, 3, 128, B).transpose(0, 2, 1, 3)
    return np.ascontiguousarray(xT)


_GATE_PERM = np.r_[H:2 * H, 0:H, 3 * H:4 * H, 2 * H:3 * H]  # [i f g o]->[f i o g]


def _prep_core(xT, wx_u, wh_u, b_u, wx_d, wh_d, b_d, reverse):
    wx_u = np.asarray(wx_u, np.float32)[:, _GATE_PERM]
    wh_u = np.asarray(wh_u, np.float32)[:, _GATE_PERM]
    b_u = np.asarray(b_u, np.float32)[_GATE_PERM]
    wx_d = np.asarray(wx_d, np.float32)[:, _GATE_PERM]
    wh_d = np.asarray(wh_d, np.float32)[:, _GATE_PERM]
    b_d = np.asarray(b_d, np.float32)[_GATE_PERM]
    wxu_pad = np.zeros((384, FOUR_H), np.float32)
    wxu_pad[:E] = wx_u
    p = np.arange(128)
    m = np.arange(T // 2)
    t_src = 2 * m[None, :] + (p[:, None] >= 64)
    t_dst = (T - 1 - t_src) if reverse else t_src
    scat = (t_dst * 64 + (p[:, None] % 64)).astype(np.int32)
    return {
        "xTu": xT,
        "wxu": np.ascontiguousarray(
            wxu_pad.reshape(3, 128, FOUR_H).transpose(1, 0, 2)),
        "whu": np.ascontiguousarray(
            np.asarray(wh_u, np.float32).reshape(4, 128, FOUR_H).transpose(1, 0, 2)),
        "bu": np.ascontiguousarray(
            np.broadcast_to(np.asarray(b_u, np.float32), (128, FOUR_H))),
        "wxd": np.ascontiguousarray(
            np.asarray(wx_d, np.float32).reshape(4, 128, FOUR_H).transpose(1, 0, 2)),
        "whd": np.ascontiguousarray(
            np.asarray(wh_d, np.float32).reshape(4, 128, FOUR_H).transpose(1, 0, 2)),
        "bd": np.ascontiguousarray(
            np.broadcast_to(np.asarray(b_d, np.float32), (128, FOUR_H))),
        "id64": np.eye(64, dtype=np.float32),
        "scat": scat,
    }


def _run(in_maps, rep=1):
    import os
    from concourse.bass_utils import run_bass_kernel_spmd
    key = f"nc{rep}"
    if key not in _CACHE:
        _CACHE[key] = _build_program(rep)
    return run_bass_kernel_spmd(_CACHE[key], in_maps, core_ids=list(range(6)))


def build_in_maps(x, uni_Wx, uni_Wh, uni_b, fwd_Wx, fwd_Wh, fwd_b,
                  bwd_Wx, bwd_Wh, bwd_b):
    xT = _prep_shared(np.asarray(x, np.float32))
    in_maps = []
    for c in range(6):
        br = c % 3
        if c < 3:
            wx_d, wh_d, b_d, rev = fwd_Wx[br], fwd_Wh[br], fwd_b[br], False
        else:
            wx_d, wh_d, b_d, rev = bwd_Wx[br], bwd_Wh[br], bwd_b[br], True
        in_maps.append(_prep_core(xT, np.asarray(uni_Wx[br], np.float32),
                                  uni_Wh[br], uni_b[br], wx_d, wh_d, b_d, rev))
    return in_maps


def kernel(x, uni_Wx, uni_Wh, uni_b, fwd_Wx, fwd_Wh, fwd_b,
           bwd_Wx, bwd_Wh, bwd_b):
    in_maps = build_in_maps(x, uni_Wx, uni_Wh, uni_b, fwd_Wx, fwd_Wh, fwd_b,
                            bwd_Wx, bwd_Wh, bwd_b)
    res = _run(in_maps)
    out = np.empty((B, NB * 2 * H), np.float32)
    for c in range(6):
        br = c % 3
        off = br * 2 * H + (0 if c < 3 else H)
        out[:, off:off + H] = res.results[c]["rmax"]
    return out

